# revision 1
# baseline (speedup 1.0000x reference)
"""Trainium2 Bass kernel for nn_DigitConvolutionalModel.

Model: x(B,784) -> reshape 28x28 -> 3x3 valid cross-correlation (kernel is an
input) -> flatten 676 -> Linear(676,128)+ReLU -> Linear(128,10).

Strategy:
  * Fold the 3x3 conv into the first linear layer on the host: the conv is a
    linear map, so h = relu(x @ W1eff.T + b1) with W1eff (128, 784) built by
    scattering conv_w-weighted copies of w1 onto the 28x28 grid. The device
    kernel is then a plain 2-layer MLP over 784 features.
  * Pure data parallelism: batch 65536 split as 8192 rows per NeuronCore,
    weights replicated.
  * The PE streams 1 column/cycle regardless of operand dtype, so layer-1 is
    7 chunk-matmuls + 1 layer-2 matmul = 8 cycles/sample = ~27.5us/core at
    2.4GHz. That is the compute floor. fp16 x would need ~36us of DMA
    (DMA-bound); full fp8 would need ~18us (PE-bound with DMA slack). So we
    ride the ridge: ship N8 of the 7 feature chunks as fp8-e3m4 (1B) and the
    rest as fp16 (2B), picking N8 so DMA time sits just under the PE floor
    while quantization error stays well below the 2e-2 gate (e3m4 keeps 4
    mantissa bits; all-e3m4 measures ~1.3e-2, N8=4 ~1.0e-2).
  * x is shipped feature-major, packed per block so every block load is one
    DMA of 112 fully contiguous per-partition runs (large descriptors run at
    HBM line rate; the baseline's 4KB descriptors capped at ~250 GB/s).
  * Engine layout: sync ring carries only x loads (never blocked), ACT does
    weight loads + the relu+bias epilogue (activation: relu(psum + b1)),
    DVE does the layer-2 bias add, gpsimd/SWDGE stores per-block outputs so
    stores never head-of-line-block the x stream; the final store goes on
    the (by then idle) sync ring to get HWDGE latency on the critical tail.
  * Block sizes: small first block so the PE starts early, 1024-wide steady
    state, small final blocks so the post-DMA tail is short.
"""

from contextlib import ExitStack

import numpy as np

B = 65536
H = W = 28
K = 3
CH = CW = 26
FEAT = H * W          # 784
HID = 128
OUT = 10
NCORES = 8
BC = B // NCORES      # 8192 rows per core

KC = 112              # contraction-chunk partition size
KCH = 7               # chunks: 7 * 112 = 784
NT = 512              # max batch rows per compute tile (one PSUM bank fp32)
X8SCALE = 2.0         # x is quantized as e3m4(2*x); 0.5 folded into w1

# variant "mixN": N chunks of x shipped as fp8-e3m4, 7-N as fp16.
# "f16" == mix0 (safe, ~5e-4 err), "f8" == mix7 (~1.3e-2 err).
VARIANT = "mix4"

_NC_CACHE = {}


def _n8(variant):
    if variant == "f16":
        return 0
    if variant == "f8":
        return KCH
    if variant.startswith("mix"):
        n = int(variant[3:])
        assert 0 <= n <= KCH
        return n
    raise ValueError(variant)


def _blocks(bc):
    if bc == 8192:
        # small first block (early PE start), uniform middle, small last
        # blocks (short tail)
        blocks = [256, 1024, 1024, 1024, 1024, 1024, 1024, 1024, 512, 256]
    else:
        step = min(1024, bc)
        blocks = [min(step, bc - o) for o in range(0, bc, step)]
    assert sum(blocks) == bc
    return blocks


def _tiles(xb):
    out, t0 = [], 0
    while t0 < xb:
        nt = min(NT, xb - t0)
        out.append((t0, nt))
        t0 += nt
    return out


def _build_nc(bc, variant):
    from concourse import bacc
    import concourse.mybir as mybir
    import concourse.tile as tile

    f32 = mybir.dt.float32
    f16 = mybir.dt.float16
    f8 = mybir.dt.float8e3
    n8 = _n8(variant)
    n16 = KCH - n8
    blocks = _blocks(bc)

    nc = bacc.Bacc(
        "TRN2",
        target_bir_lowering=False,
        debug=False,
        enable_asserts=False,
        num_devices=NCORES,
    )
    # per-partition layout inside block b (xb cols): [chunk, col] so the
    # whole block load is one contiguous run of n*xb elems per partition.
    # NOTE: partition dim must stay 112 — a 113-partition DMA defeats the
    # HWDGE descriptor spray and funnels the whole queue through ONE SDMA
    # engine (measured: 8x bandwidth collapse).
    xT8 = (
        nc.dram_tensor("xT8", [KC, n8 * bc], f8, kind="ExternalInput").ap()
        if n8
        else None
    )
    xT16 = (
        nc.dram_tensor("xT16", [KC, n16 * bc], f16, kind="ExternalInput").ap()
        if n16
        else None
    )
    w1t = nc.dram_tensor("w1t", [KC, KCH, HID], f16, kind="ExternalInput").ap()
    # w2, b2 and b1 packed as one [32, HID] tensor (rows 0-9 = w2, row 10 =
    # b2, row 11 = b1): 32 fat descriptors instead of 266 4-20B ones (each
    # tiny descriptor is a full HBM round trip), transposed on the DVE
    w2p = nc.dram_tensor("w2p", [32, HID], f16, kind="ExternalInput").ap()
    outT = nc.dram_tensor("outT", [OUT, bc], f32, kind="ExternalOutput").ap()

    with ExitStack() as ctx:
        tc = ctx.enter_context(tile.TileContext(nc))
        wpool = ctx.enter_context(tc.tile_pool(name="w", bufs=1))
        x8pool = ctx.enter_context(tc.tile_pool(name="x8", bufs=4))
        x16pool = ctx.enter_context(tc.tile_pool(name="x16", bufs=4))
        hpool = ctx.enter_context(tc.tile_pool(name="h", bufs=3))
        opool = ctx.enter_context(tc.tile_pool(name="o", bufs=4))
        p1pool = ctx.enter_context(tc.tile_pool(name="p1", bufs=5, space="PSUM"))
        p2pool = ctx.enter_context(tc.tile_pool(name="p2", bufs=3, space="PSUM"))

        # w1 rides the sync ring ahead of the x8 stream; chunk 0 is its own
        # small DMA so the very first matmul doesn't wait on all 7 chunks
        w1s = wpool.tile([KC, KCH, HID], f16)
        nc.sync.dma_start(w1s[:, 0:1, :], w1t[:, 0:1, :])
        nc.sync.dma_start(w1s[:, 1:KCH, :], w1t[:, 1:KCH, :])
        w2ps = wpool.tile([32, HID], f16)
        nc.scalar.dma_start(w2ps[:], w2p[:])
        w2sT = wpool.tile([HID, 32], f16)
        for j in range(HID // 32):
            # DVE transpose flips one 32x32 block; stitch the full transpose
            nc.vector.transpose(
                w2sT[32 * j : 32 * (j + 1), 0:32], w2ps[0:32, 32 * j : 32 * (j + 1)]
            )
        w2l = w2sT[:, 0:OUT]        # layer-2 lhsT [128, 10]
        bvec = wpool.tile([HID, 2], f32)  # tensor_scalar wants f32 scalars
        nc.vector.tensor_copy(bvec[:], w2sT[:, OUT : OUT + 2])
        b2s = bvec[0:OUT, 0:1]      # b2 as per-partition scalar [10, 1]
        b1s = bvec[:, 1:2]          # b1 as per-partition scalar [128, 1]

        # PE warm-up: the HAM clock gate holds the PE at 1.2 GHz until it
        # has seen ~3.4us of sustained activity. Burn dummy matmuls on
        # scratch during the first x block's DMA so the real stream starts
        # at 2.4 GHz. Garbage operands are fine: start=True overwrites
        # PSUM, and the first real matmul overwrites it again.
        warm = wpool.tile([KC, 640], f16)
        nc.vector.memset(warm[:], 0.0)
        pwarm = p1pool.tile([HID, NT], f32, tag="p1", name="p1_warm")
        for i in range(6):
            nc.tensor.matmul(
                pwarm[:], warm[:, :HID], warm[:, HID : HID + NT],
                start=True, stop=True, skip_group_check=True,
            )

        # global tile list: (blk_idx, block_off, xb, t0, nt, first_of_block)
        gtiles = []
        off = 0
        for bi, xb in enumerate(blocks):
            for ti, (t0, nt) in enumerate(_tiles(xb)):
                gtiles.append((bi, off, xb, t0, nt, ti == 0))
            off += xb

        xs8 = [None] * len(blocks)
        xs16 = [None] * len(blocks)
        os_ = [None] * len(blocks)
        done_tiles = [0] * len(blocks)  # epilogues emitted per block
        ntiles_of = [len(_tiles(xb)) for xb in blocks]
        # software pipeline: the L2 matmul of tile j-2 is emitted while the
        # PE chews on tile j's layer-1, so the ACT relu (emitted right after
        # tile j-2's layer-1) has two full tiles of slack before the PE
        # needs its output
        pend = []  # [(p1 tile, bi, t0, nt), ...]

        add = mybir.AluOpType.add
        mx = mybir.AluOpType.max

        def flush_one():
            p1, bi, t0, nt = pend.pop(0)
            hs = hpool.tile([HID, nt], f16, tag="hs", name=f"hs_{bi}_{t0}")
            nc.vector.tensor_scalar(hs[:], p1[:], b1s, 0.0, add, mx)
            p2 = p2pool.tile([OUT, nt], f32, tag="p2", name=f"p2_{bi}_{t0}")
            nc.tensor.matmul(p2[:], w2l, hs[:], start=True, stop=True)
            nc.vector.tensor_scalar_add(os_[bi][:, t0 : t0 + nt], p2[:], b2s)
            done_tiles[bi] += 1
            # store a fully finished block via SWDGE (never blocks x loads)
            if done_tiles[bi] == ntiles_of[bi] and bi < len(blocks) - 1:
                boff = sum(blocks[:bi])
                nc.gpsimd.dma_start(outT[:, boff : boff + blocks[bi]], os_[bi][:])
                os_[bi] = None

        for bi, boff, xb, t0, nt, first in gtiles:
            if first:
                # per-size tags: mixing tile sizes in one tag ring makes the
                # pool heap overlap buffers and serialize on stale readers
                if xT8 is not None:
                    xs8[bi] = x8pool.tile(
                        [KC, n8 * xb], f8, tag=f"xs8_{xb}", name=f"xs8_{bi}"
                    )
                    nc.sync.dma_start(
                        xs8[bi][:], xT8[:, n8 * boff : n8 * (boff + xb)]
                    )
                if xT16 is not None:
                    # x16 rides the second HWDGE ring (ACT) so the two x
                    # streams run on two DMA queues in parallel
                    xs16[bi] = x16pool.tile(
                        [KC, n16 * xb], f16, tag=f"xs16_{xb}", name=f"xs16_{bi}"
                    )
                    nc.scalar.dma_start(
                        xs16[bi][:], xT16[:, n16 * boff : n16 * (boff + xb)]
                    )
                os_[bi] = opool.tile([OUT, xb], f32, tag=f"os_{xb}", name=f"os_{bi}")

            # emit the relu for the tile the ACT ring should do next, BEFORE
            # this tile's matmuls, so its dma_start successor can't delay it
            if len(pend) >= 2:
                flush_one()

            p1 = p1pool.tile([HID, nt], f32, tag="p1", name=f"p1_{bi}_{t0}")
            for c in range(n8):
                nc.tensor.matmul(
                    p1[:],
                    w1s[:, c, :],
                    xs8[bi][:, c * xb + t0 : c * xb + t0 + nt],
                    start=(c == 0),
                    stop=(c == KCH - 1),
                )
            for c in range(n16):
                nc.tensor.matmul(
                    p1[:],
                    w1s[:, n8 + c, :],
                    xs16[bi][:, c * xb + t0 : c * xb + t0 + nt],
                    start=(n8 == 0 and c == 0),
                    stop=(n8 + c == KCH - 1),
                )
            pend.append((p1, bi, t0, nt))

        while pend:
            flush_one()
        off = 0
        for bi, xb in enumerate(blocks):
            if os_[bi] is not None:
                # the final stores go on the (now idle) sync ring: HWDGE
                # latency on the critical tail
                nc.sync.dma_start(outT[:, off : off + xb], os_[bi][:])
                os_[bi] = None
            off += xb

    nc.compile()
    return nc


def get_nc(bc=BC, variant=VARIANT):
    key = (bc, variant)
    if key not in _NC_CACHE:
        _NC_CACHE[key] = _build_nc(bc, variant)
    return _NC_CACHE[key]


def _pack_blocked(xr, blocks, np_dt):
    """[nch, kc, bc] chunk-split shard -> [kc, nch*bc] block-packed layout."""
    nch, kc, bc = xr.shape
    out = np.empty((kc, nch * bc), dtype=np_dt)
    off = 0
    for xb in blocks:
        blk = xr[:, :, off : off + xb]  # [nch, kc, xb]
        out[:, nch * off : nch * (off + xb)] = (
            blk.transpose(1, 0, 2).reshape(kc, nch * xb)
        )
        off += xb
    return out


def _host_prep(x, conv_w, w1, b1, w2, b2, variant):
    """Fold conv into layer-1 weights and lay out per-core device inputs."""
    import ml_dtypes

    n8 = _n8(variant)
    n16 = KCH - n8

    x = np.asarray(x, dtype=np.float32)
    conv_w = np.asarray(conv_w, dtype=np.float32)
    w1 = np.asarray(w1, dtype=np.float32)
    b1 = np.asarray(b1, dtype=np.float32)
    w2 = np.asarray(w2, dtype=np.float32)
    b2 = np.asarray(b2, dtype=np.float32)

    w1_img = w1.reshape(HID, CH, CW)
    w1eff = np.zeros((HID, H, W), dtype=np.float32)
    for di in range(K):
        for dj in range(K):
            w1eff[:, di : di + CH, dj : dj + CW] += conv_w[di, dj] * w1_img
    w1eff = w1eff.reshape(HID, FEAT)

    # [784,128] -> [7,112,128] -> [112,7,128]; fp8 chunks carry the folded
    # 1/X8SCALE so the device dequant is free
    w1full = w1eff.T.reshape(KCH, KC, HID).copy()
    w1full[:n8] *= 1.0 / X8SCALE
    w1t_host = np.ascontiguousarray(w1full.transpose(1, 0, 2)).astype(np.float16)
    # rows 0-9: w2; row 10: b2; row 11: b1 (device reads them back out of
    # the DVE-transposed tile as per-partition scalar columns)
    w2p_host = np.zeros((32, HID), dtype=np.float16)
    w2p_host[:OUT] = w2.astype(np.float16)
    w2p_host[OUT, :OUT] = b2.astype(np.float16)
    w2p_host[OUT + 1] = b1.astype(np.float16)

    blocks = _blocks(BC)
    in_maps = []
    for c in range(NCORES):
        shardT = x[c * BC : (c + 1) * BC].T  # [784, BC] view
        xr = np.ascontiguousarray(shardT).reshape(KCH, KC, BC)
        im = {"w1t": w1t_host, "w2p": w2p_host}
        if n8:
            q8 = (xr[:n8] * X8SCALE).astype(ml_dtypes.float8_e3m4)
            im["xT8"] = _pack_blocked(q8, blocks, ml_dtypes.float8_e3m4)
        if n16:
            q16 = xr[n8:].astype(np.float16)
            im["xT16"] = _pack_blocked(q16, blocks, np.float16)
        in_maps.append(im)
    return in_maps


def run(x, conv_w, w1, b1, w2, b2, trace=False, variant=VARIANT):
    from concourse.bass_utils import run_bass_kernel_spmd

    in_maps = _host_prep(x, conv_w, w1, b1, w2, b2, variant)
    nc = get_nc(BC, variant)
    res = run_bass_kernel_spmd(nc, in_maps, list(range(NCORES)), trace=trace)
    outT = np.concatenate([r["outT"] for r in res.results], axis=1)  # [10, B]
    return np.ascontiguousarray(outT.T), res


def kernel(x, conv_w, w1, b1, w2, b2):
    out, _ = run(x, conv_w, w1, b1, w2, b2)
    return out



# revision 2
# speedup vs baseline: 1.1301x; 1.1301x over previous
"""Trainium2 Bass kernel for nn_DigitConvolutionalModel.

Model: x(B,784) -> reshape 28x28 -> 3x3 valid cross-correlation (kernel is an
input) -> flatten 676 -> Linear(676,128)+ReLU -> Linear(128,10).

Strategy:
  * Fold the 3x3 conv into the first linear layer on the host: the conv is a
    linear map, so h = relu(x @ W1eff.T + b1) with W1eff (128, 784) built by
    scattering conv_w-weighted copies of w1 onto the 28x28 grid. The device
    kernel is then a plain 2-layer MLP over 784 features.
  * Pure data parallelism: batch 65536 split as 8192 rows per NeuronCore,
    weights replicated.
  * The PE streams 1 column/cycle regardless of operand dtype (fp8 gets no
    2x here: DoubleRow perf mode needs e4m3/e5m2 on BOTH operands, whose
    3-bit mantissa would blow the 2e-2 error gate). Layer-1 is 7
    chunk-matmuls + 1 layer-2 matmul = 8 cycles/sample = ~27.3us/core at
    2.4GHz. That is the compute floor.
  * The 16 SDMA engines run ~15-19 GB/s each (~245-300 GB/s aggregate) at
    any descriptor size >= 256B, so DMA time is proportional to bytes:
    fp16 x = 12.8MB = ~45us (DMA-bound), mix4 = 9.4MB = ~38us, full
    e3m4 = 6.4MB = ~24us which rides just under the PE floor. e3m4 keeps
    4 mantissa bits; all-e3m4 measures ~1.3e-2 against the 2e-2 gate.
  * x is shipped feature-major as TWO dram tensors so the single fp8
    stream can ride BOTH HWDGE rings in parallel (one queue caps at
    ~140 GB/s): chunks 0-3 (xA) on the sync ring, chunks 4-6 (xB) on the
    ACT ring. Per block the per-partition runs are fully contiguous.
  * Engine layout: sync ring carries xA + w1 (x block 0 issued FIRST so
    the PE can start ASAP), ACT ring carries xB + the packed w2/b tensor,
    DVE does relu+bias epilogues, gpsimd/SWDGE stores per-block outputs so
    stores never head-of-line-block the x stream; the final store goes on
    the (by then idle) sync ring.
  * The HAM clock gate holds the PE at 1.2 GHz until it has seen ~5us of
    sustained activity, and a PE stall resets the ramp. So: dummy warm-up
    matmuls bridge the window between the framework preamble and x block
    0's arrival, and the block schedule is sized so the PE never starves
    mid-stream (small blocks while the PE is cold, bigger once it's hot,
    DMA slightly ahead of the PE throughout).
"""

from contextlib import ExitStack

import numpy as np

B = 65536
H = W = 28
K = 3
CH = CW = 26
FEAT = H * W          # 784
HID = 128
OUT = 10
NCORES = 8
BC = B // NCORES      # 8192 rows per core

KC = 112              # contraction-chunk partition size
KCH = 7               # chunks: 7 * 112 = 784
NA = 4                # chunks 0-3 ride the sync ring
NB = KCH - NA         # chunks 4-6 ride the ACT ring
NT = 512              # max batch rows per compute tile (one PSUM bank fp32)
NWARM = 4             # 512-col dummy matmuls bridging preamble -> block 0
X8SCALE = 2.0         # x is quantized as e3m4(2*x); 0.5 folded into w1

# variant "mixN": N chunks of x shipped as fp8-e3m4, 7-N as fp16.
# "f16" == mix0 (safe, ~5e-4 err), "f8" == mix7 (~1.3e-2 err).
VARIANT = "f8"

_NC_CACHE = {}


def _n8(variant):
    if variant == "f16":
        return 0
    if variant == "f8":
        return KCH
    if variant.startswith("mix"):
        n = int(variant[3:])
        assert 0 <= n <= KCH
        return n
    raise ValueError(variant)


def _blocks(bc):
    if bc == 8192:
        # small blocks while the PE is cold (1.2GHz), bigger once hot;
        # keeps DMA ~0.2-0.4us ahead of the PE at every block boundary
        blocks = [256, 512, 1024, 1280, 1280, 1280, 1024, 1024, 512]
    else:
        step = min(1024, bc)
        blocks = [min(step, bc - o) for o in range(0, bc, step)]
    assert sum(blocks) == bc
    return blocks


def _tiles(xb):
    out, t0 = [], 0
    while t0 < xb:
        nt = min(NT, xb - t0)
        out.append((t0, nt))
        t0 += nt
    return out


def _build_nc(bc, variant):
    from concourse import bacc
    import concourse.mybir as mybir
    import concourse.tile as tile

    f32 = mybir.dt.float32
    f16 = mybir.dt.float16
    f8 = mybir.dt.float8e3
    n8 = _n8(variant)
    # chunk c dtype: first n8 chunks fp8, rest fp16
    cdt = [f8 if c < n8 else f16 for c in range(KCH)]
    csz = [1 if c < n8 else 2 for c in range(KCH)]
    awid = sum(csz[:NA])   # bytes per column, chunks 0-3
    bwid = sum(csz[NA:])   # bytes per column, chunks 4-6
    blocks = _blocks(bc)

    nc = bacc.Bacc(
        "TRN2",
        target_bir_lowering=False,
        debug=False,
        enable_asserts=False,
        num_devices=NCORES,
    )
    # per-partition layout inside block b (xb cols): [chunk, col] so the
    # whole block load is one contiguous run of ~wid*xb bytes per partition.
    # NOTE: partition dim must stay 112 — a 113-partition DMA defeats the
    # HWDGE descriptor spray and funnels the whole queue through ONE SDMA
    # engine (measured: 8x bandwidth collapse).
    xA = nc.dram_tensor("xA", [KC, awid * bc], mybir.dt.uint8, kind="ExternalInput").ap()
    xB = nc.dram_tensor("xB", [KC, bwid * bc], mybir.dt.uint8, kind="ExternalInput").ap()
    w1t = nc.dram_tensor("w1t", [KC, KCH, HID], f16, kind="ExternalInput").ap()
    # w2, b2 and b1 packed as one [32, HID] tensor (rows 0-9 = w2, row 10 =
    # b2, row 11 = b1): 32 fat descriptors instead of 266 4-20B ones (each
    # tiny descriptor is a full HBM round trip), transposed on the DVE
    w2p = nc.dram_tensor("w2p", [32, HID], f16, kind="ExternalInput").ap()
    outT = nc.dram_tensor("outT", [OUT, bc], f32, kind="ExternalOutput").ap()

    with ExitStack() as ctx:
        tc = ctx.enter_context(tile.TileContext(nc))
        wpool = ctx.enter_context(tc.tile_pool(name="w", bufs=1))
        xapool = ctx.enter_context(tc.tile_pool(name="xa", bufs=4))
        xbpool = ctx.enter_context(tc.tile_pool(name="xb", bufs=4))
        hpool = ctx.enter_context(tc.tile_pool(name="h", bufs=3))
        opool = ctx.enter_context(tc.tile_pool(name="o", bufs=4))
        p1pool = ctx.enter_context(tc.tile_pool(name="p1", bufs=5, space="PSUM"))
        p2pool = ctx.enter_context(tc.tile_pool(name="p2", bufs=3, space="PSUM"))

        # global tile list: (blk_idx, block_off, xb, t0, nt, first_of_block)
        gtiles = []
        off = 0
        for bi, xb in enumerate(blocks):
            for ti, (t0, nt) in enumerate(_tiles(xb)):
                gtiles.append((bi, off, xb, t0, nt, ti == 0))
            off += xb

        xsA = [None] * len(blocks)
        xsB = [None] * len(blocks)
        os_ = [None] * len(blocks)

        # x block 0 is issued before anything else on both rings so the
        # first real matmul can start as early as possible
        xb0 = blocks[0]
        xsA[0] = xapool.tile([KC, awid * xb0], mybir.dt.uint8, tag=f"xsA_{xb0}", name="xsA_0")
        nc.sync.dma_start(xsA[0][:], xA[:, 0 : awid * xb0])
        xsB[0] = xbpool.tile([KC, bwid * xb0], mybir.dt.uint8, tag=f"xsB_{xb0}", name="xsB_0")
        nc.scalar.dma_start(xsB[0][:], xB[:, 0 : bwid * xb0])

        # w1 rides the sync ring behind x block 0; chunk 0 is its own small
        # DMA so the very first matmul doesn't wait on all 7 chunks
        w1s = wpool.tile([KC, KCH, HID], f16)
        nc.sync.dma_start(w1s[:, 0:1, :], w1t[:, 0:1, :])
        nc.sync.dma_start(w1s[:, 1:KCH, :], w1t[:, 1:KCH, :])
        w2ps = wpool.tile([32, HID], f16)
        nc.scalar.dma_start(w2ps[:], w2p[:])
        w2sT = wpool.tile([HID, 32], f16)
        for j in range(HID // 32):
            # DVE transpose flips one 32x32 block; stitch the full transpose
            nc.vector.transpose(
                w2sT[32 * j : 32 * (j + 1), 0:32], w2ps[0:32, 32 * j : 32 * (j + 1)]
            )
        w2l = w2sT[:, 0:OUT]        # layer-2 lhsT [128, 10]
        bvec = wpool.tile([HID, 2], f32)  # tensor_scalar wants f32 scalars
        nc.vector.tensor_copy(bvec[:], w2sT[:, OUT : OUT + 2])
        b2s = bvec[0:OUT, 0:1]      # b2 as per-partition scalar [10, 1]
        b1s = bvec[:, 1:2]          # b1 as per-partition scalar [128, 1]

        # PE warm-up: the HAM clock gate holds the PE at 1.2 GHz until it
        # has seen ~5us of sustained activity, and a PE stall resets the
        # ramp. Burn dummy matmuls on scratch during block 0's DMA so the
        # PE never idles between the framework preamble and the real
        # stream. Garbage operands are fine: start=True overwrites PSUM,
        # and the first real matmul overwrites it again.
        warm = wpool.tile([KC, 640], f16)
        nc.vector.memset(warm[:], 0.0)
        pwarm = p1pool.tile([HID, NT], f32, tag="p1", name="p1_warm")
        for i in range(NWARM):
            nc.tensor.matmul(
                pwarm[:], warm[:, :HID], warm[:, HID : HID + NT],
                start=True, stop=True, skip_group_check=True,
            )

        done_tiles = [0] * len(blocks)  # epilogues emitted per block
        ntiles_of = [len(_tiles(xb)) for xb in blocks]
        # software pipeline: the L2 matmul of tile j-2 is emitted while the
        # PE chews on tile j's layer-1, so the DVE relu (emitted right after
        # tile j-2's layer-1) has two full tiles of slack before the PE
        # needs its output
        pend = []  # [(p1 tile, bi, t0, nt), ...]

        add = mybir.AluOpType.add
        mx = mybir.AluOpType.max

        def flush_one():
            p1, bi, t0, nt = pend.pop(0)
            hs = hpool.tile([HID, nt], f16, tag="hs", name=f"hs_{bi}_{t0}")
            nc.vector.tensor_scalar(hs[:], p1[:], b1s, 0.0, add, mx)
            p2 = p2pool.tile([OUT, nt], f32, tag="p2", name=f"p2_{bi}_{t0}")
            nc.tensor.matmul(p2[:], w2l, hs[:], start=True, stop=True)
            nc.vector.tensor_scalar_add(os_[bi][:, t0 : t0 + nt], p2[:], b2s)
            done_tiles[bi] += 1
            # store a fully finished block via SWDGE (never blocks x loads)
            if done_tiles[bi] == ntiles_of[bi] and bi < len(blocks) - 1:
                boff = sum(blocks[:bi])
                nc.gpsimd.dma_start(outT[:, boff : boff + blocks[bi]], os_[bi][:])
                os_[bi] = None

        def chunk_rhs(bi, xb, c, t0, nt):
            # chunk c's columns t0:t0+nt inside block bi, as dtype cdt[c]
            if c < NA:
                t, pre = xsA[bi], sum(csz[:c])
            else:
                t, pre = xsB[bi], sum(csz[NA:c])
            base = pre * xb
            if csz[c] == 1:
                return t[:, base + t0 : base + t0 + nt].bitcast(f8)
            return t[:, base + 2 * t0 : base + 2 * (t0 + nt)].bitcast(f16)

        for bi, boff, xb, t0, nt, first in gtiles:
            if first:
                # per-size tags: mixing tile sizes in one tag ring makes the
                # pool heap overlap buffers and serialize on stale readers
                if bi > 0:
                    xsA[bi] = xapool.tile(
                        [KC, awid * xb], mybir.dt.uint8, tag=f"xsA_{xb}", name=f"xsA_{bi}"
                    )
                    nc.sync.dma_start(
                        xsA[bi][:], xA[:, awid * boff : awid * (boff + xb)]
                    )
                    xsB[bi] = xbpool.tile(
                        [KC, bwid * xb], mybir.dt.uint8, tag=f"xsB_{xb}", name=f"xsB_{bi}"
                    )
                    nc.scalar.dma_start(
                        xsB[bi][:], xB[:, bwid * boff : bwid * (boff + xb)]
                    )
                os_[bi] = opool.tile([OUT, xb], f32, tag=f"os_{xb}", name=f"os_{bi}")

            # emit the relu for the tile the DVE should do next, BEFORE
            # this tile's matmuls, so its dma_start successor can't delay it
            if len(pend) >= 2:
                flush_one()

            p1 = p1pool.tile([HID, nt], f32, tag="p1", name=f"p1_{bi}_{t0}")
            for c in range(KCH):
                nc.tensor.matmul(
                    p1[:],
                    w1s[:, c, :],
                    chunk_rhs(bi, xb, c, t0, nt),
                    start=(c == 0),
                    stop=(c == KCH - 1),
                )
            pend.append((p1, bi, t0, nt))

        while pend:
            flush_one()
        off = 0
        for bi, xb in enumerate(blocks):
            if os_[bi] is not None:
                # the final stores go on the (now idle) sync ring: HWDGE
                # latency on the critical tail
                nc.sync.dma_start(outT[:, off : off + xb], os_[bi][:])
                os_[bi] = None
            off += xb

    nc.compile()
    return nc


def get_nc(bc=BC, variant=VARIANT):
    key = (bc, variant)
    if key not in _NC_CACHE:
        _NC_CACHE[key] = _build_nc(bc, variant)
    return _NC_CACHE[key]


def _pack_blocked(xr, csz, blocks):
    """[nch, kc, bc] chunk-split shard (already per-chunk quantized to byte
    rows: fp8 chunks are 1 row of bytes, fp16 chunks 2) -> [kc, W*bc]
    block-packed uint8 layout."""
    kc = xr[0].shape[0]
    bc = xr[0].shape[1] // csz[0] if csz else 0
    wid = sum(csz)
    out = np.empty((kc, wid * bc), dtype=np.uint8)
    off = 0
    for xb in blocks:
        dst = off * wid
        for ci, xc in enumerate(xr):
            w = csz[ci]
            blk = xc[:, w * off : w * (off + xb)]
            out[:, dst : dst + w * xb] = blk
            dst += w * xb
        off += xb
    return out


def _host_prep(x, conv_w, w1, b1, w2, b2, variant):
    """Fold conv into layer-1 weights and lay out per-core device inputs."""
    import ml_dtypes

    n8 = _n8(variant)

    x = np.asarray(x, dtype=np.float32)
    conv_w = np.asarray(conv_w, dtype=np.float32)
    w1 = np.asarray(w1, dtype=np.float32)
    b1 = np.asarray(b1, dtype=np.float32)
    w2 = np.asarray(w2, dtype=np.float32)
    b2 = np.asarray(b2, dtype=np.float32)

    w1_img = w1.reshape(HID, CH, CW)
    w1eff = np.zeros((HID, H, W), dtype=np.float32)
    for di in range(K):
        for dj in range(K):
            w1eff[:, di : di + CH, dj : dj + CW] += conv_w[di, dj] * w1_img
    w1eff = w1eff.reshape(HID, FEAT)

    # [784,128] -> [7,112,128] -> [112,7,128]; fp8 chunks carry the folded
    # 1/X8SCALE so the device dequant is free
    w1full = w1eff.T.reshape(KCH, KC, HID).copy()
    w1full[:n8] *= 1.0 / X8SCALE
    w1t_host = np.ascontiguousarray(w1full.transpose(1, 0, 2)).astype(np.float16)
    # rows 0-9: w2; row 10: b2; row 11: b1 (device reads them back out of
    # the DVE-transposed tile as per-partition scalar columns)
    w2p_host = np.zeros((32, HID), dtype=np.float16)
    w2p_host[:OUT] = w2.astype(np.float16)
    w2p_host[OUT, :OUT] = b2.astype(np.float16)
    w2p_host[OUT + 1] = b1.astype(np.float16)

    csz = [1 if c < n8 else 2 for c in range(KCH)]
    blocks = _blocks(BC)
    in_maps = []
    for c in range(NCORES):
        shardT = x[c * BC : (c + 1) * BC].T  # [784, BC] view
        xr = np.ascontiguousarray(shardT).reshape(KCH, KC, BC)
        rows = []
        for ci in range(KCH):
            if csz[ci] == 1:
                q = (xr[ci] * X8SCALE).astype(ml_dtypes.float8_e3m4)
                rows.append(q.view(np.uint8))
            else:
                q = xr[ci].astype(np.float16)
                rows.append(np.ascontiguousarray(q).view(np.uint8).reshape(KC, 2 * BC))
        im = {
            "w1t": w1t_host,
            "w2p": w2p_host,
            "xA": _pack_blocked(rows[:NA], csz[:NA], blocks),
            "xB": _pack_blocked(rows[NA:], csz[NA:], blocks),
        }
        in_maps.append(im)
    return in_maps


def run(x, conv_w, w1, b1, w2, b2, trace=False, variant=VARIANT):
    from concourse.bass_utils import run_bass_kernel_spmd

    in_maps = _host_prep(x, conv_w, w1, b1, w2, b2, variant)
    nc = get_nc(BC, variant)
    res = run_bass_kernel_spmd(nc, in_maps, list(range(NCORES)), trace=trace)
    outT = np.concatenate([r["outT"] for r in res.results], axis=1)  # [10, B]
    return np.ascontiguousarray(outT.T), res


def kernel(x, conv_w, w1, b1, w2, b2):
    out, _ = run(x, conv_w, w1, b1, w2, b2)
    return out
